# revision 18
# baseline (speedup 1.0000x reference)
"""AWQ int4 dequant + matmul (M=4096, K=4096, N=11008) on 8 TRN2 NeuronCores.

Column-parallel: qweight/scales/qzeros/bias sharded along N (1376 per core),
x replicated. Per core: dequantize the weight shard to bf16 on-chip (resident
in SBUF), transpose x tiles via one batched DMA-transpose per m-tile, bf16
matmuls with fp32 PSUM accumulation, add bias, write the output shard.

DMA dispatch is split between the two HWDGE engines (SP and ACT) because the
SP sequencer costs ~1us per instruction and serializes.
"""

import sys

if "/opt/trn_rl_repo" not in sys.path:
    sys.path.insert(0, "/opt/trn_rl_repo")

import ml_dtypes
import numpy as np

import concourse.bass as bass
import concourse.mybir as mybir
import concourse.tile as tile
from concourse import bacc, bass_utils

# Problem shapes (hardcoded per contract)
M = 4096
K = 4096
N = 11008
G = 128  # AWQ group size
N_CORES = 8
NS = N // N_CORES  # 1376 output columns per core
CS = NS // 8  # 172 packed int32 columns per core
NCH = K // 128  # 32 k-chunks (each exactly one AWQ group)
# bit-nibble i of a packed int32 holds logical column INV[i] (mod 8)
INV = [0, 2, 4, 6, 1, 3, 5, 7]
N_TILES = [(0, 512), (512, 512), (1024, 352)]

BF16 = mybir.dt.bfloat16
F32 = mybir.dt.float32
I32 = mybir.dt.int32


def build_program(m_tiles=M // 128):
    """Build the per-core Bass program (SPMD: same program, per-core shards)."""
    nc = bacc.Bacc("TRN2", target_bir_lowering=False, debug=False, num_devices=N_CORES)

    Xd = nc.dram_tensor("x", [m_tiles * 128, K], F32, kind="ExternalInput").ap()
    QWd = nc.dram_tensor("qw", [K, CS], I32, kind="ExternalInput").ap()
    Sd = nc.dram_tensor("s_bf", [NCH, NS], BF16, kind="ExternalInput").ap()
    ZSd = nc.dram_tensor("zs_bf", [NCH, NS], BF16, kind="ExternalInput").ap()
    Bd = nc.dram_tensor("bias", [1, NS], F32, kind="ExternalInput").ap()
    Od = nc.dram_tensor("out", [m_tiles * 128, NS], F32, kind="ExternalOutput").ap()

    with tile.TileContext(nc) as tc:
        with (
            tc.tile_pool(name="wpool", bufs=1) as wpool,
            tc.tile_pool(name="meta", bufs=1) as meta,
            tc.tile_pool(name="qpool", bufs=2) as qpool,
            tc.tile_pool(name="qip", bufs=2) as qip,
            tc.tile_pool(name="bcast", bufs=2) as bcast,
            tc.tile_pool(name="xf", bufs=2) as xfp,
            tc.tile_pool(name="xb", bufs=2) as xbp,
            tc.tile_pool(name="xt", bufs=2) as xtp,
            tc.tile_pool(name="op", bufs=2) as outp,
            tc.tile_pool(name="ps", bufs=8, space="PSUM") as psp,
        ):
            # Resident dequantized weights [128k, chunk, n]
            W = wpool.tile([128, NCH, NS], BF16)
            bias_bc = meta.tile([128, NS], F32)
            nc.sync.dma_start(bias_bc[:], Bd.to_broadcast([128, NS]))

            # x staging pipeline (dispatched from ACT; transpose on SP)
            H = K // 2

            def x_pipeline(mt):
                xb = xbp.tile([128, K], BF16, tag="xb", name="xb")
                xT = xtp.tile([128, NCH, 128], BF16, tag="xT", name="xT")
                for h in range(2):
                    xf = xfp.tile([128, H], F32, tag="xf", name="xf")
                    nc.scalar.dma_start(
                        xf[:], Xd[mt * 128 : (mt + 1) * 128, h * H : (h + 1) * H]
                    )
                    nc.scalar.copy(xb[:, h * H : (h + 1) * H], xf[:])
                    nc.sync.dma_start_transpose(
                        xT[:, h * (NCH // 2) : (h + 1) * (NCH // 2), :],
                        xb[:, h * H : (h + 1) * H],
                    )
                return xT

            # emit the first m-tile's x pipeline before dequant so the PE
            # can start as soon as chunk 0 of W is ready
            xT0 = x_pipeline(0)

            # Phase A: dequant all chunks, two chunks per pass (amortizes
            # per-instruction overhead of the strided nibble extracts)
            for g in range(0, NCH, 2):
                qwt = qpool.tile([128, 2, CS], I32, tag="qwt", name="qwt")
                nc.sync.dma_start(
                    qwt[:],
                    QWd[g * 128 : (g + 2) * 128, :].rearrange(
                        "(two p) c -> p two c", p=128
                    ),
                )
                qint = qip.tile([128, 2, NS], I32, tag="qint", name="qint")
                for i in range(8):
                    nc.vector.tensor_scalar(
                        qint[:, :, INV[i] :: 8],
                        qwt[:],
                        4 * i,
                        0xF,
                        mybir.AluOpType.logical_shift_right,
                        mybir.AluOpType.bitwise_and,
                    )
                for j in range(2):
                    wg = W[:, g + j, :]
                    sbc = bcast.tile([128, NS], BF16, tag="sbc", name="sbc")
                    zbc = bcast.tile([128, NS], BF16, tag="zbc", name="zbc")
                    nc.sync.dma_start(
                        sbc[:], Sd[g + j : g + j + 1, :].to_broadcast([128, NS])
                    )
                    nc.sync.dma_start(
                        zbc[:], ZSd[g + j : g + j + 1, :].to_broadcast([128, NS])
                    )
                    nc.vector.tensor_tensor(
                        wg, qint[:, j, :], sbc[:], mybir.AluOpType.mult
                    )
                    nc.vector.tensor_tensor(wg, wg, zbc[:], mybir.AluOpType.subtract)

            # Phase B: stream x tiles, cast, transpose, matmul
            for mt in range(m_tiles):
                xT = xT0 if mt == 0 else x_pipeline(mt)
                ot = outp.tile([128, NS], F32, tag="ot")
                for n0, nsz in N_TILES:
                    pt_full = psp.tile([128, 512], F32, tag="pt", name="pt")
                    pt = pt_full[:, :nsz]
                    for g in range(NCH):
                        nc.tensor.matmul(
                            pt,
                            xT[:, g, :],
                            W[:, g, n0 : n0 + nsz],
                            start=(g == 0),
                            stop=(g == NCH - 1),
                        )
                    nc.vector.tensor_tensor(
                        ot[:, n0 : n0 + nsz], pt, bias_bc[:, n0 : n0 + nsz],
                        mybir.AluOpType.add,
                    )
                nc.scalar.dma_start(Od[mt * 128 : (mt + 1) * 128, :], ot[:])

    nc.compile()
    return nc


def shard_inputs(x, qweight, scales, qzeros, bias, m_tiles=M // 128):
    """Host-side sharding + tiny preprocessing (qzeros unpack, bf16 casts)."""
    # unpack qzeros [NCH, N//8] -> z_int [NCH, N] in logical column order
    shifts = np.array([0, 16, 4, 20, 8, 24, 12, 28], dtype=np.int32)  # 4*AWQ_ORDER
    z_int = ((qzeros[:, :, None] >> shifts[None, None, :]) & 0xF).reshape(NCH, N)
    zs = (z_int.astype(np.float32) * scales).astype(ml_dtypes.bfloat16)
    s_bf = scales.astype(ml_dtypes.bfloat16)
    xm = np.ascontiguousarray(x[: m_tiles * 128])
    in_maps = []
    for c in range(N_CORES):
        nsl = slice(c * NS, (c + 1) * NS)
        in_maps.append(
            {
                "x": xm,
                "qw": np.ascontiguousarray(qweight[:, c * CS : (c + 1) * CS]),
                "s_bf": np.ascontiguousarray(s_bf[:, nsl]),
                "zs_bf": np.ascontiguousarray(zs[:, nsl]),
                "bias": np.ascontiguousarray(bias[nsl]).reshape(1, NS),
            }
        )
    return in_maps


_CACHED_NC = None


def get_program():
    global _CACHED_NC
    if _CACHED_NC is None:
        _CACHED_NC = build_program()
    return _CACHED_NC


def kernel(x, qweight, scales, qzeros, bias):
    nc = get_program()
    in_maps = shard_inputs(x, qweight, scales, qzeros, bias)
    res = bass_utils.run_bass_kernel_spmd(nc, in_maps, core_ids=list(range(N_CORES)))
    out = np.concatenate([res.results[c]["out"] for c in range(N_CORES)], axis=1)
    return out.astype(np.float32, copy=False)


# revision 19
# speedup vs baseline: 1.1499x; 1.1499x over previous
"""AWQ int4 dequant + matmul (M=4096, K=4096, N=11008) on 8 TRN2 NeuronCores.

Column-parallel: qweight/scales/qzeros/bias sharded along N (1376 per core),
x replicated. Per core: dequantize the weight shard to bf16 on-chip (resident
in SBUF), transpose x tiles via one batched DMA-transpose per m-tile, bf16
matmuls with fp32 PSUM accumulation, add bias, write the output shard.

DMA dispatch is split between the two HWDGE engines (SP and ACT) because the
SP sequencer costs ~1us per instruction and serializes.
"""

import sys

if "/opt/trn_rl_repo" not in sys.path:
    sys.path.insert(0, "/opt/trn_rl_repo")

import ml_dtypes
import numpy as np

import concourse.bass as bass
import concourse.mybir as mybir
import concourse.tile as tile
from concourse import bacc, bass_utils

# Problem shapes (hardcoded per contract)
M = 4096
K = 4096
N = 11008
G = 128  # AWQ group size
N_CORES = 8
NS = N // N_CORES  # 1376 output columns per core
CS = NS // 8  # 172 packed int32 columns per core
NCH = K // 128  # 32 k-chunks (each exactly one AWQ group)
# bit-nibble i of a packed int32 holds logical column INV[i] (mod 8)
INV = [0, 2, 4, 6, 1, 3, 5, 7]
N_TILES = [(0, 512), (512, 512), (1024, 352)]

BF16 = mybir.dt.bfloat16
F32 = mybir.dt.float32
I32 = mybir.dt.int32


def build_program(m_tiles=M // 128):
    """Build the per-core Bass program (SPMD: same program, per-core shards)."""
    nc = bacc.Bacc("TRN2", target_bir_lowering=False, debug=False, num_devices=N_CORES)

    Xd = nc.dram_tensor("x", [m_tiles * 128, K], F32, kind="ExternalInput").ap()
    QWd = nc.dram_tensor("qw", [K, CS], I32, kind="ExternalInput").ap()
    Sd = nc.dram_tensor("s_bf", [NCH, NS], BF16, kind="ExternalInput").ap()
    ZSd = nc.dram_tensor("zs_bf", [NCH, NS], BF16, kind="ExternalInput").ap()
    Bd = nc.dram_tensor("bias", [1, NS], F32, kind="ExternalInput").ap()
    Od = nc.dram_tensor("out", [m_tiles * 128, NS], F32, kind="ExternalOutput").ap()

    with tile.TileContext(nc) as tc:
        with (
            tc.tile_pool(name="wpool", bufs=1) as wpool,
            tc.tile_pool(name="meta", bufs=1) as meta,
            tc.tile_pool(name="qpool", bufs=2) as qpool,
            tc.tile_pool(name="qip", bufs=2) as qip,
            tc.tile_pool(name="bcast", bufs=2) as bcast,
            tc.tile_pool(name="xf", bufs=2) as xfp,
            tc.tile_pool(name="xb", bufs=2) as xbp,
            tc.tile_pool(name="xt", bufs=2) as xtp,
            tc.tile_pool(name="op", bufs=2) as outp,
            tc.tile_pool(name="ps", bufs=8, space="PSUM") as psp,
        ):
            # Resident dequantized weights [128k, chunk, n]
            W = wpool.tile([128, NCH, NS], BF16)
            bias_bc = meta.tile([128, NS], F32)
            nc.sync.dma_start(bias_bc[:], Bd.to_broadcast([128, NS]))

            # x staging pipeline (dispatched from ACT; transpose on SP)
            H = K // 2

            def x_pipeline(mt):
                xb = xbp.tile([128, K], BF16, tag="xb", name="xb")
                for h in range(2):
                    xf = xfp.tile([128, H], F32, tag="xf", name="xf")
                    nc.scalar.dma_start(
                        xf[:], Xd[mt * 128 : (mt + 1) * 128, h * H : (h + 1) * H]
                    )
                    nc.scalar.copy(xb[:, h * H : (h + 1) * H], xf[:])
                xT = xtp.tile([128, NCH, 128], BF16, tag="xT", name="xT")
                nc.sync.dma_start_transpose(xT[:], xb[:])
                return xT

            # emit the first m-tile's x pipeline before dequant so the PE
            # can start as soon as chunk 0 of W is ready
            xT0 = x_pipeline(0)

            # Phase A: dequant all chunks, two chunks per pass (amortizes
            # per-instruction overhead of the strided nibble extracts)
            for g in range(0, NCH, 2):
                qwt = qpool.tile([128, 2, CS], I32, tag="qwt", name="qwt")
                nc.sync.dma_start(
                    qwt[:],
                    QWd[g * 128 : (g + 2) * 128, :].rearrange(
                        "(two p) c -> p two c", p=128
                    ),
                )
                qint = qip.tile([128, 2, NS], I32, tag="qint", name="qint")
                for i in range(8):
                    nc.vector.tensor_scalar(
                        qint[:, :, INV[i] :: 8],
                        qwt[:],
                        4 * i,
                        0xF,
                        mybir.AluOpType.logical_shift_right,
                        mybir.AluOpType.bitwise_and,
                    )
                for j in range(2):
                    wg = W[:, g + j, :]
                    sbc = bcast.tile([128, NS], BF16, tag="sbc", name="sbc")
                    zbc = bcast.tile([128, NS], BF16, tag="zbc", name="zbc")
                    nc.sync.dma_start(
                        sbc[:], Sd[g + j : g + j + 1, :].to_broadcast([128, NS])
                    )
                    nc.sync.dma_start(
                        zbc[:], ZSd[g + j : g + j + 1, :].to_broadcast([128, NS])
                    )
                    nc.vector.tensor_tensor(
                        wg, qint[:, j, :], sbc[:], mybir.AluOpType.mult
                    )
                    nc.vector.tensor_tensor(wg, wg, zbc[:], mybir.AluOpType.subtract)

            # Phase B: stream x tiles, cast, transpose, matmul
            for mt in range(m_tiles):
                xT = xT0 if mt == 0 else x_pipeline(mt)
                ot = outp.tile([128, NS], F32, tag="ot")
                for n0, nsz in N_TILES:
                    pt_full = psp.tile([128, 512], F32, tag="pt", name="pt")
                    pt = pt_full[:, :nsz]
                    for g in range(NCH):
                        nc.tensor.matmul(
                            pt,
                            xT[:, g, :],
                            W[:, g, n0 : n0 + nsz],
                            start=(g == 0),
                            stop=(g == NCH - 1),
                        )
                    nc.vector.tensor_tensor(
                        ot[:, n0 : n0 + nsz], pt, bias_bc[:, n0 : n0 + nsz],
                        mybir.AluOpType.add,
                    )
                nc.scalar.dma_start(Od[mt * 128 : (mt + 1) * 128, :], ot[:])

    nc.compile()
    return nc


def shard_inputs(x, qweight, scales, qzeros, bias, m_tiles=M // 128):
    """Host-side sharding + tiny preprocessing (qzeros unpack, bf16 casts)."""
    # unpack qzeros [NCH, N//8] -> z_int [NCH, N] in logical column order
    shifts = np.array([0, 16, 4, 20, 8, 24, 12, 28], dtype=np.int32)  # 4*AWQ_ORDER
    z_int = ((qzeros[:, :, None] >> shifts[None, None, :]) & 0xF).reshape(NCH, N)
    zs = (z_int.astype(np.float32) * scales).astype(ml_dtypes.bfloat16)
    s_bf = scales.astype(ml_dtypes.bfloat16)
    xm = np.ascontiguousarray(x[: m_tiles * 128])
    in_maps = []
    for c in range(N_CORES):
        nsl = slice(c * NS, (c + 1) * NS)
        in_maps.append(
            {
                "x": xm,
                "qw": np.ascontiguousarray(qweight[:, c * CS : (c + 1) * CS]),
                "s_bf": np.ascontiguousarray(s_bf[:, nsl]),
                "zs_bf": np.ascontiguousarray(zs[:, nsl]),
                "bias": np.ascontiguousarray(bias[nsl]).reshape(1, NS),
            }
        )
    return in_maps


_CACHED_NC = None


def get_program():
    global _CACHED_NC
    if _CACHED_NC is None:
        _CACHED_NC = build_program()
    return _CACHED_NC


def kernel(x, qweight, scales, qzeros, bias):
    nc = get_program()
    in_maps = shard_inputs(x, qweight, scales, qzeros, bias)
    res = bass_utils.run_bass_kernel_spmd(nc, in_maps, core_ids=list(range(N_CORES)))
    out = np.concatenate([res.results[c]["out"] for c in range(N_CORES)], axis=1)
    return out.astype(np.float32, copy=False)
